# revision 9
# baseline (speedup 1.0000x reference)
"""Trainium2 Bass kernel for nn_CausalGraphGenerator.

Reference semantics: the per-channel conv predictor is channel-separable, so
the influence matrix A[b] is diagonal. Hence A - A^T == 0 identically and

    adj[b, i, j] = relu(0 - h) = max(-h, 0)   for i != j
    adj[b, i, i] = 0

for ANY X / conv weights. The output depends only on the scalar threshold h.
(Verified numerically against the reference, including h < 0 and perturbed X.)

Device kernel (SPMD on 8 NeuronCores, batch-parallel: core b produces batch
b's [C, C] adjacency slice):
    out = max(negmask * h, 0)
with negmask = -(1 - I) and h packed into one [C, C+1] input (col 0 = h
replicated per partition — the per-partition scalar operand of a single
tensor_scalar instruction; cols 1..C = negmask). Since negmask ∈ {-1, 0},
max(negmask * h, 0) == (1 - I) * relu(-h) exactly, in one op.

Raw Bass (no TileContext): everything runs on the Pool (gpsimd) queue —
DMA in -> wait -> tensor_scalar -> DMA out — with a single semaphore. This
avoids Tile's kernel-tail drain, whose >2 sem waits the neuronx-cc CoreV3
codegen used by the bass2jax/PJRT path rejects ("Too many sync wait
commands"), and skips Tile's all-engine barrier epilogue entirely.
"""

import numpy as np

_B, _W, _C = 4, 2048, 64
_N_CORES = 8

_CACHE = {}


def _build_nc():
    """Build (once) the single-core Bass program run SPMD on all 8 cores."""
    if "nc" in _CACHE:
        return _CACHE["nc"]

    import concourse.bass as bass
    import concourse.mybir as mybir

    nc = bass.Bass("TRN2", target_bir_lowering=False)

    packed_t = nc.dram_tensor(
        "packed", [_C, _C + 1], mybir.dt.float32, kind="ExternalInput"
    )
    out_t = nc.dram_tensor("out", [_C, _C], mybir.dt.float32, kind="ExternalOutput")

    with (
        nc.sbuf_tensor("pk", [_C, _C + 1], mybir.dt.float32) as pk,
        nc.sbuf_tensor("o", [_C, _C], mybir.dt.float32) as o,
        nc.semaphore("dma_sem") as dma_sem,
        nc.semaphore("comp_sem") as comp_sem,
        nc.Block() as block,
    ):

        @block.gpsimd
        def _(gpsimd):
            gpsimd.dma_start(out=pk[:, :], in_=packed_t.ap()).then_inc(dma_sem, 16)
            gpsimd.wait_ge(dma_sem, 16)
            gpsimd.tensor_scalar(
                out=o[:, :],
                in0=pk[:, 1 : _C + 1],
                scalar1=pk[:, 0:1],
                scalar2=0.0,
                op0=mybir.AluOpType.mult,
                op1=mybir.AluOpType.max,
            ).then_inc(comp_sem, 1)
            gpsimd.wait_ge(comp_sem, 1)
            gpsimd.dma_start(out=out_t.ap(), in_=o[:, :]).then_inc(dma_sem, 16)

    _CACHE["nc"] = nc
    return nc


def _in_map(h_value):
    hv = np.float32(np.asarray(h_value).reshape(()))
    packed = np.empty((_C, _C + 1), dtype=np.float32)
    packed[:, 0] = hv
    packed[:, 1:] = -(1.0 - np.eye(_C, dtype=np.float32))
    return {"packed": packed}


def run(h, trace=False):
    """Run the SPMD kernel on cores 0-7; returns (out [B,C,C], BassKernelResults)."""
    from concourse.bass_utils import run_bass_kernel_spmd

    nc = _build_nc()
    in_maps = [_in_map(h) for _ in range(_N_CORES)]
    res = run_bass_kernel_spmd(nc, in_maps, list(range(_N_CORES)), trace=trace)
    # Batch-parallel gather: batch b comes from core b.
    out = np.stack([res.results[b]["out"] for b in range(_B)], axis=0)
    return np.ascontiguousarray(out, dtype=np.float32), res


def kernel(X, w1, b1, w2, b2, h):
    out, _ = run(h)
    return out


# revision 11
# speedup vs baseline: 1.3090x; 1.3090x over previous
"""Trainium2 Bass kernel for nn_CausalGraphGenerator.

Reference semantics: the per-channel conv predictor is channel-separable, so
the influence matrix A[b] is diagonal. Hence A - A^T == 0 identically and

    adj[b, i, j] = relu(0 - h) = max(-h, 0)   for i != j
    adj[b, i, i] = 0

for ANY X / conv weights — the output depends only on the scalar threshold h.
(Verified numerically against the reference, including h < 0 and perturbed X.)

Device kernel (SPMD on 8 NeuronCores, batch-parallel: core b produces batch
b's [C, C] adjacency slice):
    out = max(negmask * h, 0)
with negmask = -(1 - I) and h packed into one [C, C+1] input (col 0 = h
replicated per partition — the per-partition scalar operand of a single
VectorE tensor_scalar instruction; cols 1..C = negmask). Since
negmask ∈ {-1, 0}, max(negmask * h, 0) == (1 - I) * relu(-h) exactly.

Raw Bass (no TileContext): HWDGE DMAs issue from the ACT queue, the one
tensor_scalar runs on DVE, with sem waits attached directly to the consuming
instructions. This avoids Tile's kernel-tail drain (whose >2 sem waits the
neuronx-cc CoreV3 codegen used by the bass2jax/PJRT path rejects: "Too many
sync wait commands") and Tile's all-engine barrier epilogue. Measured
~12 us/core on HW, ~3 us of which is the kernel body (the rest is fixed BSP
preamble: engine-start barrier, base-register loads, drains).
"""

import numpy as np

_B, _W, _C = 4, 2048, 64
_N_CORES = 8

_CACHE = {}


def _build_nc():
    """Build (once) the single-core Bass program run SPMD on all 8 cores."""
    if "nc" in _CACHE:
        return _CACHE["nc"]

    import concourse.bass as bass
    import concourse.mybir as mybir

    nc = bass.Bass("TRN2", target_bir_lowering=False)

    packed_t = nc.dram_tensor(
        "packed", [_C, _C + 1], mybir.dt.float32, kind="ExternalInput"
    )
    out_t = nc.dram_tensor("out", [_C, _C], mybir.dt.float32, kind="ExternalOutput")

    with (
        nc.sbuf_tensor("pk", [_C, _C + 1], mybir.dt.float32) as pk,
        nc.sbuf_tensor("o", [_C, _C], mybir.dt.float32) as o,
        nc.semaphore("dma_sem") as dma_sem,
        nc.semaphore("comp_sem") as comp_sem,
        nc.Block() as block,
    ):

        @block.scalar
        def _(act):
            act.dma_start(
                out=pk[:, :], in_=packed_t.ap(), single_packet=True
            ).then_inc(dma_sem, 16)
            act.dma_start(
                out=out_t.ap(), in_=o[:, :], single_packet=True
            )._wait_ge(comp_sem, 1).then_inc(dma_sem, 16)

        @block.vector
        def _(dve):
            dve.tensor_scalar(
                out=o[:, :],
                in0=pk[:, 1 : _C + 1],
                scalar1=pk[:, 0:1],
                scalar2=0.0,
                op0=mybir.AluOpType.mult,
                op1=mybir.AluOpType.max,
            )._wait_ge(dma_sem, 16).then_inc(comp_sem, 1)

    _CACHE["nc"] = nc
    return nc


def _in_map(h_value):
    hv = np.float32(np.asarray(h_value).reshape(()))
    packed = np.empty((_C, _C + 1), dtype=np.float32)
    packed[:, 0] = hv
    packed[:, 1:] = -(1.0 - np.eye(_C, dtype=np.float32))
    return {"packed": packed}


def _cached_exec():
    """One-time jit of the SPMD executable (same lowering as
    bass2jax.run_bass_via_pjrt's multi-core path); repeat kernel() calls
    then skip re-tracing and go straight to device execution."""
    if "exec" in _CACHE:
        return _CACHE["exec"]

    import jax
    import concourse.mybir as mybir
    from jax.sharding import Mesh, PartitionSpec
    from jax.experimental.shard_map import shard_map
    from concourse.bass2jax import (
        _bass_exec_p,
        install_neuronx_cc_hook,
        partition_id_tensor,
    )

    nc = _build_nc()
    install_neuronx_cc_hook()
    assert nc.dbg_addr is None
    partition_name = nc.partition_id_tensor.name if nc.partition_id_tensor else None

    in_names, out_names, out_avals, zero_outs = [], [], [], []
    for alloc in nc.m.functions[0].allocations:
        if not isinstance(alloc, mybir.MemoryLocationSet):
            continue
        name = alloc.memorylocations[0].name
        if alloc.kind == "ExternalInput":
            if name != partition_name:
                in_names.append(name)
        elif alloc.kind == "ExternalOutput":
            shape = tuple(alloc.tensor_shape)
            dtype = mybir.dt.np(alloc.dtype)
            out_names.append(name)
            out_avals.append(jax.core.ShapedArray(shape, dtype))
            zero_outs.append(np.zeros(shape, dtype))
    n_params = len(in_names)
    all_names = in_names + out_names + ([partition_name] if partition_name else [])

    def _body(*args):
        operands = list(args)
        if partition_name is not None:
            operands.append(partition_id_tensor())
        return tuple(
            _bass_exec_p.bind(
                *operands,
                out_avals=tuple(out_avals),
                in_names=tuple(all_names),
                out_names=tuple(out_names),
                lowering_input_output_aliases=(),
                sim_require_finite=True,
                sim_require_nnan=True,
                nc=nc,
            )
        )

    devices = jax.devices()[:_N_CORES]
    mesh = Mesh(np.asarray(devices), ("core",))
    n_outs = len(out_names)
    sharded = jax.jit(
        shard_map(
            _body,
            mesh=mesh,
            in_specs=(PartitionSpec("core"),) * (n_params + n_outs),
            out_specs=(PartitionSpec("core"),) * n_outs,
            check_rep=False,
        ),
        donate_argnums=tuple(range(n_params, n_params + n_outs)),
        keep_unused=True,
    )

    def run_spmd(in_maps):
        concat_in = [
            np.concatenate([m[name] for m in in_maps], axis=0) for name in in_names
        ]
        concat_zero = [
            np.zeros((_N_CORES * z.shape[0], *z.shape[1:]), z.dtype)
            for z in zero_outs
        ]
        out_arrs = sharded(*concat_in, *concat_zero)
        return [
            {
                name: np.asarray(out_arrs[i]).reshape(
                    _N_CORES, *out_avals[i].shape
                )[c]
                for i, name in enumerate(out_names)
            }
            for c in range(_N_CORES)
        ]

    _CACHE["exec"] = run_spmd
    return run_spmd


def run(h, trace=False):
    """Run the SPMD kernel on cores 0-7; returns (out [B,C,C], results)."""
    in_maps = [_in_map(h) for _ in range(_N_CORES)]
    if trace:
        from concourse.bass_utils import run_bass_kernel_spmd

        res = run_bass_kernel_spmd(
            _build_nc(), in_maps, list(range(_N_CORES)), trace=True
        )
        results = res.results
    else:
        res = None
        results = _cached_exec()(in_maps)
    # Batch-parallel gather: batch b comes from core b.
    out = np.stack([results[b]["out"] for b in range(_B)], axis=0)
    return np.ascontiguousarray(out, dtype=np.float32), res


def kernel(X, w1, b1, w2, b2, h):
    out, _ = run(h)
    return out


# revision 13
# speedup vs baseline: 1.3908x; 1.0625x over previous
"""Trainium2 Bass kernel for nn_CausalGraphGenerator.

Reference semantics: the per-channel conv predictor is channel-separable, so
the influence matrix A[b] is diagonal. Hence A - A^T == 0 identically and

    adj[b, i, j] = relu(0 - h) = max(-h, 0)   for i != j
    adj[b, i, i] = 0

for ANY X / conv weights — the output depends only on the scalar threshold h.
(Verified numerically against the reference, including h < 0 and perturbed X.)

Device kernel (SPMD on 8 NeuronCores, batch-parallel: core b produces batch
b's [C, C] adjacency slice):
    out = max(negmask * h, 0)
with negmask = -(1 - I) and h packed into one [C, C+1] input (col 0 = h
replicated per partition — the per-partition scalar operand of a single
VectorE tensor_scalar instruction; cols 1..C = negmask). Since
negmask ∈ {-1, 0}, max(negmask * h, 0) == (1 - I) * relu(-h) exactly.

Raw Bass (no TileContext, no Block): HWDGE DMAs issue from the ACT queue, the
one tensor_scalar runs on DVE, with sem waits attached directly to the
consuming instructions. This avoids Tile's kernel-tail drain (whose >2 sem
waits the neuronx-cc CoreV3 codegen used by the bass2jax/PJRT path rejects:
"Too many sync wait commands"), Tile's all-engine barrier epilogue, and the
Block-exit barrier. Validated in CoreSim (race detector) and on HW across
repeated executions with varying h (semaphores are reset per execution by the
runtime). Measured ~11.6 us/core on HW, ~3 us of which is the kernel body
(the rest is fixed BSP preamble: engine-start barrier, base-register loads).
"""

import numpy as np

_B, _W, _C = 4, 2048, 64
_N_CORES = 8

_CACHE = {}


def _build_nc():
    """Build (once) the single-core Bass program run SPMD on all 8 cores."""
    if "nc" in _CACHE:
        return _CACHE["nc"]

    import concourse.bass as bass
    import concourse.mybir as mybir

    nc = bass.Bass("TRN2", target_bir_lowering=False)

    packed_t = nc.dram_tensor(
        "packed", [_C, _C + 1], mybir.dt.float32, kind="ExternalInput"
    )
    out_t = nc.dram_tensor("out", [_C, _C], mybir.dt.float32, kind="ExternalOutput")

    with (
        nc.sbuf_tensor("pk", [_C, _C + 1], mybir.dt.float32) as pk,
        nc.sbuf_tensor("o", [_C, _C], mybir.dt.float32) as o,
        nc.semaphore("dma_sem") as dma_sem,
        nc.semaphore("comp_sem") as comp_sem,
    ):
        nc.scalar.dma_start(
            out=pk[:, :], in_=packed_t.ap(), single_packet=True
        ).then_inc(dma_sem, 16)
        nc.vector.tensor_scalar(
            out=o[:, :],
            in0=pk[:, 1 : _C + 1],
            scalar1=pk[:, 0:1],
            scalar2=0.0,
            op0=mybir.AluOpType.mult,
            op1=mybir.AluOpType.max,
        )._wait_ge(dma_sem, 16).then_inc(comp_sem, 1)
        nc.scalar.dma_start(
            out=out_t.ap(), in_=o[:, :], single_packet=True
        )._wait_ge(comp_sem, 1).then_inc(dma_sem, 16)

    _CACHE["nc"] = nc
    return nc


def _in_map(h_value):
    hv = np.float32(np.asarray(h_value).reshape(()))
    packed = np.empty((_C, _C + 1), dtype=np.float32)
    packed[:, 0] = hv
    packed[:, 1:] = -(1.0 - np.eye(_C, dtype=np.float32))
    return {"packed": packed}


def _cached_exec():
    """One-time jit of the SPMD executable (same lowering as
    bass2jax.run_bass_via_pjrt's multi-core path); repeat kernel() calls
    then skip re-tracing and go straight to device execution."""
    if "exec" in _CACHE:
        return _CACHE["exec"]

    import jax
    import concourse.mybir as mybir
    from jax.sharding import Mesh, PartitionSpec
    from jax.experimental.shard_map import shard_map
    from concourse.bass2jax import (
        _bass_exec_p,
        install_neuronx_cc_hook,
        partition_id_tensor,
    )

    nc = _build_nc()
    install_neuronx_cc_hook()
    assert nc.dbg_addr is None
    partition_name = nc.partition_id_tensor.name if nc.partition_id_tensor else None

    in_names, out_names, out_avals, zero_outs = [], [], [], []
    for alloc in nc.m.functions[0].allocations:
        if not isinstance(alloc, mybir.MemoryLocationSet):
            continue
        name = alloc.memorylocations[0].name
        if alloc.kind == "ExternalInput":
            if name != partition_name:
                in_names.append(name)
        elif alloc.kind == "ExternalOutput":
            shape = tuple(alloc.tensor_shape)
            dtype = mybir.dt.np(alloc.dtype)
            out_names.append(name)
            out_avals.append(jax.core.ShapedArray(shape, dtype))
            zero_outs.append(np.zeros(shape, dtype))
    n_params = len(in_names)
    all_names = in_names + out_names + ([partition_name] if partition_name else [])

    def _body(*args):
        operands = list(args)
        if partition_name is not None:
            operands.append(partition_id_tensor())
        return tuple(
            _bass_exec_p.bind(
                *operands,
                out_avals=tuple(out_avals),
                in_names=tuple(all_names),
                out_names=tuple(out_names),
                lowering_input_output_aliases=(),
                sim_require_finite=True,
                sim_require_nnan=True,
                nc=nc,
            )
        )

    devices = jax.devices()[:_N_CORES]
    mesh = Mesh(np.asarray(devices), ("core",))
    n_outs = len(out_names)
    sharded = jax.jit(
        shard_map(
            _body,
            mesh=mesh,
            in_specs=(PartitionSpec("core"),) * (n_params + n_outs),
            out_specs=(PartitionSpec("core"),) * n_outs,
            check_rep=False,
        ),
        donate_argnums=tuple(range(n_params, n_params + n_outs)),
        keep_unused=True,
    )

    def run_spmd(in_maps):
        concat_in = [
            np.concatenate([m[name] for m in in_maps], axis=0) for name in in_names
        ]
        concat_zero = [
            np.zeros((_N_CORES * z.shape[0], *z.shape[1:]), z.dtype)
            for z in zero_outs
        ]
        out_arrs = sharded(*concat_in, *concat_zero)
        return [
            {
                name: np.asarray(out_arrs[i]).reshape(
                    _N_CORES, *out_avals[i].shape
                )[c]
                for i, name in enumerate(out_names)
            }
            for c in range(_N_CORES)
        ]

    _CACHE["exec"] = run_spmd
    return run_spmd


def run(h, trace=False):
    """Run the SPMD kernel on cores 0-7; returns (out [B,C,C], results)."""
    in_maps = [_in_map(h) for _ in range(_N_CORES)]
    if trace:
        from concourse.bass_utils import run_bass_kernel_spmd

        res = run_bass_kernel_spmd(
            _build_nc(), in_maps, list(range(_N_CORES)), trace=True
        )
        results = res.results
    else:
        res = None
        results = _cached_exec()(in_maps)
    # Batch-parallel gather: batch b comes from core b.
    out = np.stack([results[b]["out"] for b in range(_B)], axis=0)
    return np.ascontiguousarray(out, dtype=np.float32), res


def kernel(X, w1, b1, w2, b2, h):
    out, _ = run(h)
    return out


# revision 15
# speedup vs baseline: 1.4298x; 1.0281x over previous
"""Trainium2 Bass kernel for nn_CausalGraphGenerator.

Reference semantics: the per-channel conv predictor is channel-separable, so
the influence matrix A[b] is diagonal. Hence A - A^T == 0 identically and

    adj[b, i, j] = relu(0 - h) = max(-h, 0)   for i != j
    adj[b, i, i] = 0

for ANY X / conv weights — the output depends only on the scalar threshold h.
(Verified numerically against the reference, including h < 0 and perturbed X.)

Device kernel (SPMD on 8 NeuronCores, batch-parallel: core b produces batch
b's [C, C] adjacency slice):
    out = max(negmask * h, 0)
with negmask = -(1 - I) and h packed into one [C, C+1] input (col 0 = h
replicated per partition — the per-partition scalar operand of a single
VectorE tensor_scalar instruction; cols 1..C = negmask). Since
negmask ∈ {-1, 0}, max(negmask * h, 0) == (1 - I) * relu(-h) exactly.

Raw Bass (no TileContext, no Block): HWDGE DMAs issue from the ACT queue, the
one tensor_scalar runs on DVE, with sem waits attached directly to the
consuming instructions. This avoids Tile's kernel-tail drain (whose >2 sem
waits the neuronx-cc CoreV3 codegen used by the bass2jax/PJRT path rejects:
"Too many sync wait commands"), Tile's all-engine barrier epilogue, and the
Block-exit barrier. Validated in CoreSim (race detector) and on HW across
repeated executions with varying h (semaphores are reset per execution by the
runtime). Measured ~11.6 us/core on HW, ~3 us of which is the kernel body
(the rest is fixed BSP preamble: engine-start barrier, base-register loads).
"""

import numpy as np

_B, _W, _C = 4, 2048, 64
_N_CORES = 8

_CACHE = {}


def _build_nc():
    """Build (once) the single-core Bass program run SPMD on all 8 cores."""
    if "nc" in _CACHE:
        return _CACHE["nc"]

    import concourse.bass as bass
    import concourse.mybir as mybir

    nc = bass.Bass("TRN2", target_bir_lowering=False)

    packed_t = nc.dram_tensor(
        "packed", [_C, _C + 1], mybir.dt.float32, kind="ExternalInput"
    )
    out_t = nc.dram_tensor("out", [_C, _C], mybir.dt.float32, kind="ExternalOutput")

    with (
        nc.sbuf_tensor("pk", [_C, _C + 1], mybir.dt.float32) as pk,
        nc.sbuf_tensor("o", [_C, _C], mybir.dt.float32) as o,
        nc.semaphore("dma_sem") as dma_sem,
        nc.semaphore("comp_sem") as comp_sem,
    ):
        nc.scalar.dma_start(
            out=pk[:, :], in_=packed_t.ap(), single_packet=True
        ).then_inc(dma_sem, 16)
        nc.vector.tensor_scalar(
            out=o[:, :],
            in0=pk[:, 1 : _C + 1],
            scalar1=pk[:, 0:1],
            scalar2=0.0,
            op0=mybir.AluOpType.mult,
            op1=mybir.AluOpType.max,
        )._wait_ge(dma_sem, 16).then_inc(comp_sem, 1)
        nc.scalar.dma_start(
            out=out_t.ap(), in_=o[:, :], single_packet=True
        )._wait_ge(comp_sem, 1).then_inc(dma_sem, 16)

    _CACHE["nc"] = nc
    return nc


def _in_map(h_value):
    hv = np.float32(np.asarray(h_value).reshape(()))
    packed = np.empty((_C, _C + 1), dtype=np.float32)
    packed[:, 0] = hv
    packed[:, 1:] = -(1.0 - np.eye(_C, dtype=np.float32))
    return {"packed": packed}


def _cached_exec():
    """One-time jit of the SPMD executable (same lowering as
    bass2jax.run_bass_via_pjrt's multi-core path); repeat kernel() calls
    then skip re-tracing and go straight to device execution."""
    if "exec" in _CACHE:
        return _CACHE["exec"]

    import jax
    import concourse.mybir as mybir
    from jax.sharding import Mesh, PartitionSpec
    from jax.experimental.shard_map import shard_map
    from concourse.bass2jax import (
        _bass_exec_p,
        install_neuronx_cc_hook,
        partition_id_tensor,
    )

    nc = _build_nc()
    install_neuronx_cc_hook()
    assert nc.dbg_addr is None
    partition_name = nc.partition_id_tensor.name if nc.partition_id_tensor else None

    in_names, out_names, out_avals, zero_outs = [], [], [], []
    for alloc in nc.m.functions[0].allocations:
        if not isinstance(alloc, mybir.MemoryLocationSet):
            continue
        name = alloc.memorylocations[0].name
        if alloc.kind == "ExternalInput":
            if name != partition_name:
                in_names.append(name)
        elif alloc.kind == "ExternalOutput":
            shape = tuple(alloc.tensor_shape)
            dtype = mybir.dt.np(alloc.dtype)
            out_names.append(name)
            out_avals.append(jax.core.ShapedArray(shape, dtype))
            zero_outs.append(np.zeros(shape, dtype))
    n_params = len(in_names)
    all_names = in_names + out_names + ([partition_name] if partition_name else [])

    def _body(*args):
        operands = list(args)
        if partition_name is not None:
            operands.append(partition_id_tensor())
        return tuple(
            _bass_exec_p.bind(
                *operands,
                out_avals=tuple(out_avals),
                in_names=tuple(all_names),
                out_names=tuple(out_names),
                lowering_input_output_aliases=(),
                sim_require_finite=True,
                sim_require_nnan=True,
                nc=nc,
            )
        )

    devices = jax.devices()[:_N_CORES]
    mesh = Mesh(np.asarray(devices), ("core",))
    n_outs = len(out_names)
    sharded = jax.jit(
        shard_map(
            _body,
            mesh=mesh,
            in_specs=(PartitionSpec("core"),) * (n_params + n_outs),
            out_specs=(PartitionSpec("core"),) * n_outs,
            check_rep=False,
        ),
        donate_argnums=tuple(range(n_params, n_params + n_outs)),
        keep_unused=True,
    )

    def run_spmd(in_maps):
        concat_in = [
            np.concatenate([m[name] for m in in_maps], axis=0) for name in in_names
        ]
        concat_zero = [
            np.zeros((_N_CORES * z.shape[0], *z.shape[1:]), z.dtype)
            for z in zero_outs
        ]
        out_arrs = sharded(*concat_in, *concat_zero)
        return [
            {
                name: np.asarray(out_arrs[i]).reshape(
                    _N_CORES, *out_avals[i].shape
                )[c]
                for i, name in enumerate(out_names)
            }
            for c in range(_N_CORES)
        ]

    _CACHE["exec"] = run_spmd
    return run_spmd


def run(h, trace=False):
    """Run the SPMD kernel on cores 0-7; returns (out [B,C,C], results)."""
    in_maps = [_in_map(h) for _ in range(_N_CORES)]
    if trace:
        from concourse.bass_utils import run_bass_kernel_spmd

        res = run_bass_kernel_spmd(
            _build_nc(), in_maps, list(range(_N_CORES)), trace=True
        )
        results = res.results
    else:
        res = None
        try:
            results = _cached_exec()(in_maps)
        except Exception:  # fall back to the stock (re-tracing) runner
            _CACHE.pop("exec", None)
            from concourse.bass_utils import run_bass_kernel_spmd

            results = run_bass_kernel_spmd(
                _build_nc(), in_maps, list(range(_N_CORES))
            ).results
    # Batch-parallel gather: batch b comes from core b.
    out = np.stack([results[b]["out"] for b in range(_B)], axis=0)
    return np.ascontiguousarray(out, dtype=np.float32), res


def kernel(X, w1, b1, w2, b2, h, **_unused):
    out, _ = run(h)
    return out


# revision 18
# speedup vs baseline: 1.6223x; 1.1346x over previous
"""Trainium2 Bass kernel for nn_CausalGraphGenerator.

Reference semantics: the per-channel conv predictor is channel-separable, so
the influence matrix A[b] is diagonal. Hence A - A^T == 0 identically and

    adj[b, i, j] = relu(0 - h) = max(-h, 0)   for i != j
    adj[b, i, i] = 0

for ANY X / conv weights — the output depends only on the scalar threshold h.
(Verified numerically against the reference, including h < 0 and perturbed X.)

Device kernel (SPMD on 8 NeuronCores, batch-parallel: core b produces batch
b's [C, C] adjacency slice):
    out = max(negmask * h, 0)
with negmask = -(1 - I) and h packed into one [C, C+1] input (col 0 = h
replicated per partition — the per-partition scalar operand of a single
VectorE tensor_scalar instruction; cols 1..C = negmask). Since
negmask ∈ {-1, 0}, max(negmask * h, 0) == (1 - I) * relu(-h) exactly.

Raw Bass (no TileContext, no Block): the in-DMA issues from the ACT HWDGE
queue, the one tensor_scalar runs on DVE, the out-DMA from the SP HWDGE queue
(pre-armed on the compute semaphore), with sem waits attached directly to the
consuming instructions. This avoids Tile's kernel-tail drain (whose >2 sem
waits the neuronx-cc CoreV3 codegen used by the bass2jax/PJRT path rejects:
"Too many sync wait commands"), Tile's all-engine barrier epilogue, and the
Block-exit barrier. Bass's BIR preamble (register movs / const memsets /
all-engine barrier) is stripped after tracing — see _strip_preamble. Validated
in CoreSim (race detector) and on HW across repeated executions with varying h
(semaphores are reset per execution by the runtime). Measured ~8.3 us/core on
HW (stable +/-30 ns); ~2.4 us of that is the kernel body, the rest is fixed
walrus/BSP scaffolding (engine-start events — the PE engine's start event
arrives ~3 us late, gating the start barrier — base-register TENSOR_LOADs,
and inter-preamble barriers) that exists for any NEFF on this path.
"""

import numpy as np

_B, _W, _C = 4, 2048, 64
_N_CORES = 8

_CACHE = {}


def _build_nc():
    """Build (once) the single-core Bass program run SPMD on all 8 cores."""
    if "nc" in _CACHE:
        return _CACHE["nc"]

    import concourse.bass as bass
    import concourse.mybir as mybir

    nc = bass.Bass("TRN2", target_bir_lowering=False)

    packed_t = nc.dram_tensor(
        "packed", [_C, _C + 1], mybir.dt.float32, kind="ExternalInput"
    )
    out_t = nc.dram_tensor("out", [_C, _C], mybir.dt.float32, kind="ExternalOutput")

    with (
        nc.sbuf_tensor("pk", [_C, _C + 1], mybir.dt.float32) as pk,
        nc.sbuf_tensor("o", [_C, _C], mybir.dt.float32) as o,
        nc.semaphore("dma_sem") as dma_sem,
        nc.semaphore("comp_sem") as comp_sem,
    ):
        nc.scalar.dma_start(out=pk[:, :], in_=packed_t.ap()).then_inc(dma_sem, 16)
        nc.vector.tensor_scalar(
            out=o[:, :],
            in0=pk[:, 1 : _C + 1],
            scalar1=pk[:, 0:1],
            scalar2=0.0,
            op0=mybir.AluOpType.mult,
            op1=mybir.AluOpType.max,
        )._wait_ge(dma_sem, 16).then_inc(comp_sem, 1)
        # out-DMA on the otherwise-idle SP HWDGE queue: SP sits pre-armed on
        # comp_sem and fires the moment the tensor_scalar retires, and the
        # end-of-kernel queue drains then run on two engines in parallel
        # (measured ~160 ns faster than issuing both DMAs from ACT)
        nc.sync.dma_start(out=out_t.ap(), in_=o[:, :])._wait_ge(
            comp_sem, 1
        ).then_inc(dma_sem, 16)

    _strip_preamble(nc)
    _CACHE["nc"] = nc
    return nc


def _strip_preamble(nc):
    """Drop Bass's BIR preamble (per-engine register movs, const-AP memsets,
    and the all-engine barrier) — none of it is used by this kernel's three
    instructions (the tensor_scalar's scalar2 lowers to an immediate, not a
    const AP). Measured saving: ~3.1 us/exec (11.5 -> 8.4 us). Guarded by an
    exact structural match so a concourse layout change falls back to the
    unstripped (still correct) program. Validated in CoreSim and on HW with
    varying h across repeated executions."""
    import concourse.mybir as mybir

    bb = nc.m.functions[0].blocks[0]
    insts = list(bb.instructions)
    strippable = (
        mybir.InstRegisterMove,
        mybir.InstMemset,
        mybir.InstDrain,
        mybir.InstEventSemaphore,
    )
    if (
        len(insts) >= 5
        and isinstance(insts[0], mybir.InstCall)
        and all(isinstance(i, strippable) for i in insts[1:-3])
        and isinstance(insts[-3], mybir.InstDMACopy)
        and isinstance(insts[-2], mybir.InstTensorScalarPtr)
        and isinstance(insts[-1], mybir.InstDMACopy)
    ):
        bb.instructions = [insts[0]] + insts[-3:]


def _in_map(h_value):
    hv = np.float32(np.asarray(h_value).reshape(()))
    packed = np.empty((_C, _C + 1), dtype=np.float32)
    packed[:, 0] = hv
    packed[:, 1:] = -(1.0 - np.eye(_C, dtype=np.float32))
    return {"packed": packed}


def _cached_exec():
    """One-time jit of the SPMD executable (same lowering as
    bass2jax.run_bass_via_pjrt's multi-core path); repeat kernel() calls
    then skip re-tracing and go straight to device execution."""
    if "exec" in _CACHE:
        return _CACHE["exec"]

    import jax
    import concourse.mybir as mybir
    from jax.sharding import Mesh, PartitionSpec
    from jax.experimental.shard_map import shard_map
    from concourse.bass2jax import (
        _bass_exec_p,
        install_neuronx_cc_hook,
        partition_id_tensor,
    )

    nc = _build_nc()
    install_neuronx_cc_hook()
    assert nc.dbg_addr is None
    partition_name = nc.partition_id_tensor.name if nc.partition_id_tensor else None

    in_names, out_names, out_avals, zero_outs = [], [], [], []
    for alloc in nc.m.functions[0].allocations:
        if not isinstance(alloc, mybir.MemoryLocationSet):
            continue
        name = alloc.memorylocations[0].name
        if alloc.kind == "ExternalInput":
            if name != partition_name:
                in_names.append(name)
        elif alloc.kind == "ExternalOutput":
            shape = tuple(alloc.tensor_shape)
            dtype = mybir.dt.np(alloc.dtype)
            out_names.append(name)
            out_avals.append(jax.core.ShapedArray(shape, dtype))
            zero_outs.append(np.zeros(shape, dtype))
    n_params = len(in_names)
    all_names = in_names + out_names + ([partition_name] if partition_name else [])

    def _body(*args):
        operands = list(args)
        if partition_name is not None:
            operands.append(partition_id_tensor())
        return tuple(
            _bass_exec_p.bind(
                *operands,
                out_avals=tuple(out_avals),
                in_names=tuple(all_names),
                out_names=tuple(out_names),
                lowering_input_output_aliases=(),
                sim_require_finite=True,
                sim_require_nnan=True,
                nc=nc,
            )
        )

    devices = jax.devices()[:_N_CORES]
    mesh = Mesh(np.asarray(devices), ("core",))
    n_outs = len(out_names)
    sharded = jax.jit(
        shard_map(
            _body,
            mesh=mesh,
            in_specs=(PartitionSpec("core"),) * (n_params + n_outs),
            out_specs=(PartitionSpec("core"),) * n_outs,
            check_rep=False,
        ),
        donate_argnums=tuple(range(n_params, n_params + n_outs)),
        keep_unused=True,
    )

    def run_spmd(in_maps):
        concat_in = [
            np.concatenate([m[name] for m in in_maps], axis=0) for name in in_names
        ]
        concat_zero = [
            np.zeros((_N_CORES * z.shape[0], *z.shape[1:]), z.dtype)
            for z in zero_outs
        ]
        out_arrs = sharded(*concat_in, *concat_zero)
        return [
            {
                name: np.asarray(out_arrs[i]).reshape(
                    _N_CORES, *out_avals[i].shape
                )[c]
                for i, name in enumerate(out_names)
            }
            for c in range(_N_CORES)
        ]

    _CACHE["exec"] = run_spmd
    return run_spmd


def run(h, trace=False):
    """Run the SPMD kernel on cores 0-7; returns (out [B,C,C], results)."""
    in_maps = [_in_map(h) for _ in range(_N_CORES)]
    if trace:
        from concourse.bass_utils import run_bass_kernel_spmd

        res = run_bass_kernel_spmd(
            _build_nc(), in_maps, list(range(_N_CORES)), trace=True
        )
        results = res.results
    else:
        res = None
        try:
            results = _cached_exec()(in_maps)
        except Exception:  # fall back to the stock (re-tracing) runner
            _CACHE.pop("exec", None)
            from concourse.bass_utils import run_bass_kernel_spmd

            results = run_bass_kernel_spmd(
                _build_nc(), in_maps, list(range(_N_CORES))
            ).results
    # Batch-parallel gather: batch b comes from core b.
    out = np.stack([results[b]["out"] for b in range(_B)], axis=0)
    return np.ascontiguousarray(out, dtype=np.float32), res


def kernel(X, w1, b1, w2, b2, h, **_unused):
    out, _ = run(h)
    return out
